# revision 44
# baseline (speedup 1.0000x reference)
"""Trainium2 Bass kernel for a dense transformer block (B=2, T=2048, C=1024,
H=16, D=64, FF=4096), SPMD on 8 NeuronCores.

Sharding: pure data-parallel over tokens, zero collectives.
  core cid -> batch b = cid // 4, rank r = cid % 4.
  Each batch's 2048 tokens split into 8 causal chunks of 256; rank r owns
  chunks {r, 7-r} (zigzag balances causal attention work across ranks).
  Each core redundantly computes LN1 + K + V for its whole batch (cheaper
  than the slow on-chip ring collectives), then attention, projection, LN2
  and the MLP for its own 512 tokens only.

Layouts: matmuls keep activations transposed ([c, t]: contraction dim on
partitions); LayerNorm runs in [t, c] (free-dim reductions); PE transposes
convert. Weights arrive host-packed so every weight load is one large DMA
with multi-KB contiguous partition lines (HWDGE costs ~625ns per DMA
instruction, so instruction count dominates). K bounces through DRAM in a
per-head-contiguous [row-block, 128, T] layout. V is computed directly in
[t, d] layout (ln1T slices as lhsT, host-packed wv_rhs as rhs), with the
softmax ones-column appended, staged per 128-token block and bounced through
DRAM once — the per-head on-chip V re-transposes of earlier revisions are
gone, and attention loads all 16 V blocks into SBUF once per iteration. The
1/Z partition broadcast is a K=1 matmul. LN gamma/beta and the 1/sqrt(D) score
scale are folded into weights host-side. Matmuls run as float32r (full-rate
fp32 storage mode, ~1e-4 matmul rel err).

One NEFF runs on all 8 cores, so causal ranges must be data-independent:
attention runs a fixed union schedule (q-half 0: s-chunks 0..3, q-half 1:
s-chunks 0..7) and host-fed per-core additive masks (0 / -30000) gate
inactive chunks and the diagonal triangle; the mask is accumulated into the
score PSUM with an identity-lhsT matmul, so masking costs PE cycles instead
of Pool ones, and exp runs over a full [128, 512] bank per s-chunk. q-half 1
is always the late chunk (>= 4), so its s-chunks 0..3 are unconditionally
active and skip the mask accumulate. Q heads are packed in pairs so the Q
GEMM runs with all 128 PE rows. PE transposes land four-per-PSUM-bank and
copy out with one 512-wide Act instruction.

Pipelining note: empirically this program is bound by how many independent
per-tile dependency chains are in flight, not by PE cycles, instruction
count, or DMA bytes (halving any of those moved the time <5%). The LN work
pools and the per-head K prefetch run 3-deep for this reason — each bump
from 2 to 3 measured ~-5% wall; SBUF is the binding constraint on going
deeper.

Dtype note: all matmuls stay float32r on purpose — bf16/fp8 matmuls emit a
separate InstLdweights per matmul on this toolchain (f32r streams weights
with no preload), and the kernel is dependency/instruction-bound, not
FLOP-bound, so halving cycle counts while adding ~1800 PE instructions is a
net loss. Keeping K/V fully SBUF-resident was also evaluated: it does not
fit in 24 MB SBUF at f32r (ln1T + V + weights peak ~250 KB/partition), and
shrinking to bf16 reintroduces the Ldweights tax.
"""

import numpy as np

B, T, C = 2, 2048, 1024
H, D = 16, 64
FF = 4 * C
EPS = 1e-6
N_CORES = 8
NCHUNK = 8
CH = T // NCHUNK        # 256 tokens per causal chunk
RANKS = 4
OWN = T // RANKS        # 512 tokens owned per core
P = 128
NB = 512                # matmul moving-dim tile
KC = C // P             # 8 contraction chunks over C
TB = T // NB            # 4 column blocks over T
FB = FF // P            # 32 ff row blocks

MASKED_PAIRS = [(0, sc) for sc in range(4)] + [(1, sc) for sc in range(4, 8)]
MASKED_SET = set(MASKED_PAIRS)
SRANGE = (RANKS, NCHUNK)  # union s-chunk counts per q-half


def build_core_program(nc, tile, mybir, n_iters=1):
    from contextlib import ExitStack
    from concourse import masks as masks_mod

    dt = mybir.dt
    f32 = dt.float32
    f32r = dt.float32r
    AF = mybir.ActivationFunctionType
    ALU = mybir.AluOpType
    AX = mybir.AxisListType

    x_full = nc.dram_tensor("x_full", [T, C], f32, kind="ExternalInput").ap()
    x_own = nc.dram_tensor("x_own", [OWN, C], f32, kind="ExternalInput").ap()
    xT_own = nc.dram_tensor("xT_own", [C, OWN], f32, kind="ExternalInput").ap()
    # host-packed weights: one row-block per leading index, each a [128, X]
    # tile whose columns are (kc, j) so kc-slices serve as matmul lhsT/rhs
    wq_p = nc.dram_tensor("wq_p", [H // 2, P, KC * P], f32r,
                          kind="ExternalInput").ap()
    wkv_p = nc.dram_tensor("wkv_p", [KC, P, KC * P], f32r,
                           kind="ExternalInput").ap()
    wv_rhs = nc.dram_tensor("wv_rhs", [KC, P, C], f32r,
                            kind="ExternalInput").ap()
    wproj_p = nc.dram_tensor("wproj_p", [KC, P, KC * P], f32r,
                             kind="ExternalInput").ap()
    wl1_p = nc.dram_tensor("wl1_p", [FB, P, KC * P], f32r,
                           kind="ExternalInput").ap()
    wl3_p = nc.dram_tensor("wl3_p", [KC, P, FB * P], f32r,
                           kind="ExternalInput").ap()
    bqkv = nc.dram_tensor("bqkv", [3 * C], f32, kind="ExternalInput").ap()
    bproj = nc.dram_tensor("bproj", [C], f32, kind="ExternalInput").ap()
    bl1 = nc.dram_tensor("bl1", [FF], f32, kind="ExternalInput").ap()
    bl3 = nc.dram_tensor("bl3", [C], f32, kind="ExternalInput").ap()
    mask_in = nc.dram_tensor(
        "mask", [len(MASKED_PAIRS), 2, P, CH], f32r, kind="ExternalInput"
    ).ap()
    outT = nc.dram_tensor("outT", [C, OWN], f32, kind="ExternalOutput").ap()

    with tile.TileContext(nc) as tc, ExitStack() as ctx:
        dramp = ctx.enter_context(
            tc.tile_pool(name="dramb", bufs=1, space="DRAM")
        )
        kT_d = dramp.tile([KC, P, T], f32r, name="kT_d")   # [m, 2-heads, T]
        v65_d = dramp.tile([T // P, P, H, D + 1], f32r, name="v65_d")
        qT_d = dramp.tile([C, OWN], f32r, name="qT_d")

        cpool = ctx.enter_context(tc.tile_pool(name="const", bufs=1))
        identity = cpool.tile([P, P], f32, name="identity")
        masks_mod.make_identity(nc, identity[:])
        id_r = cpool.tile([P, P], f32r, name="id_r")
        nc.scalar.activation(id_r[:], identity[:], AF.Identity)
        onesf = cpool.tile([P, P], f32, name="onesf")
        nc.vector.memset(onesf[:], 1.0)
        ones1 = cpool.tile([1, D], f32r, name="ones1")
        nc.scalar.activation(ones1[:], onesf[0:1, 0:D], AF.Identity)
        ones1p = cpool.tile([1, P], f32r, name="ones1p")
        nc.scalar.activation(ones1p[:], onesf[0:1, :], AF.Identity)
        bv1f = cpool.tile([1, C], f32, name="bv1f")
        nc.sync.dma_start(
            bv1f[:], bqkv[2 * C:3 * C].rearrange("(o c) -> o c", o=1))
        bv1 = cpool.tile([1, C], f32r, name="bv1")
        nc.scalar.activation(bv1[:], bv1f[:], AF.Identity)

        bqkv_t = cpool.tile([P, 3 * KC], f32, name="bqkv_t")
        nc.sync.dma_start(bqkv_t[:], bqkv.rearrange("(j p) -> p j", p=P))
        bq128 = cpool.tile([P, H // 2], f32, name="bq128")
        nc.sync.dma_start(bq128[:], bqkv[0:C].rearrange("(j p) -> p j", p=P))
        bproj_t = cpool.tile([P, KC], f32, name="bproj_t")
        nc.sync.dma_start(bproj_t[:], bproj.rearrange("(j p) -> p j", p=P))
        bl1_t = cpool.tile([P, FB], f32, name="bl1_t")
        nc.sync.dma_start(bl1_t[:], bl1.rearrange("(j p) -> p j", p=P))
        bl3_t = cpool.tile([P, KC], f32, name="bl3_t")
        nc.sync.dma_start(bl3_t[:], bl3.rearrange("(j p) -> p j", p=P))

        stat = ctx.enter_context(tc.tile_pool(name="stat", bufs=4))
        evp = ctx.enter_context(tc.tile_pool(name="ev", bufs=4))
        psum = ctx.enter_context(tc.tile_pool(name="ps", bufs=3, space="PSUM"))
        pst = ctx.enter_context(tc.tile_pool(name="pst", bufs=2, space="PSUM"))
        pav = ctx.enter_context(tc.tile_pool(name="pav", bufs=2, space="PSUM"))
        pbc = ctx.enter_context(tc.tile_pool(name="pbc", bufs=1, space="PSUM"))

        def mm(out, lhsT, rhs, **kw):
            nc.tensor.matmul(out, lhsT, rhs, **kw)

        def layernorm_tile(xt, z, work):
            s = stat.tile([P, 1], f32, name="s")
            nc.vector.reduce_sum(s[:], xt[:], axis=AX.X)
            nmu = stat.tile([P, 1], f32, name="nmu")
            nc.scalar.mul(nmu[:], s[:], -1.0 / C)
            sq = work.tile([P, C], f32, name="sq")
            nc.scalar.activation(sq[:], xt[:], AF.Square, bias=nmu[:, 0:1])
            ssq = stat.tile([P, 1], f32, name="ssq")
            nc.vector.reduce_sum(ssq[:], sq[:], axis=AX.X)
            sd = stat.tile([P, 1], f32, name="sd")
            nc.scalar.activation(sd[:], ssq[:], AF.Sqrt, scale=1.0 / (C - 1))
            sde = stat.tile([P, 1], f32, name="sde")
            nc.vector.tensor_scalar_add(sde[:], sd[:], EPS)
            rs = stat.tile([P, 1], f32, name="rs")
            nc.vector.reciprocal(rs[:], sde[:])
            nc.vector.tensor_scalar(
                z[:], xt[:], nmu[:, 0:1], rs[:, 0:1], ALU.add, ALU.mult
            )

        def transpose_to(dst3, src_tile, dst_col):
            # dst3: one [P, KC, X] tile; 4 PE transposes fill one PSUM bank,
            # one 512-wide Act copy drains it.
            for g in range(KC // 4):
                ps = pst.tile([P, 4, P], f32, name="tps")
                for jj in range(4):
                    j = 4 * g + jj
                    nc.tensor.transpose(
                        ps[:, jj, :], src_tile[:, j * P:(j + 1) * P],
                        identity[:]
                    )
                nc.scalar.copy(
                    dst3[:, 4 * g:4 * (g + 1), dst_col:dst_col + P], ps[:]
                )

        def ln_transpose(src_dram, row0, dst3, dst_col, work):
            xt = work.tile([P, C], f32, name="xt")
            nc.sync.dma_start(xt[:], src_dram[row0:row0 + P, :])
            z = work.tile([P, C], f32, name="z")
            layernorm_tile(xt, z, work)
            transpose_to(dst3, z, dst_col)

        def body(ctx2):
            # ---- A-own + D: LN1 on own 512 tokens -> lnq; Q -> qT_d ----
            with tc.tile_pool(name="lnqp", bufs=1) as lnqp, \
                 tc.tile_pool(name="workq", bufs=3) as workq, \
                 tc.tile_pool(name="wqp", bufs=3) as wqp:
                lnq = lnqp.tile([P, KC, OWN], f32r, name="lnq")
                for i in range(OWN // P):
                    ln_transpose(x_own, i * P, lnq, i * P, workq)
                for m in range(H // 2):
                    wq = wqp.tile([P, KC * P], f32r, name="wq")
                    nc.sync.dma_start(wq[:], wq_p[m])
                    ps = psum.tile([P, NB], f32, name="ps")
                    for kc in range(KC):
                        mm(ps[:], wq[:, kc * P:(kc + 1) * P], lnq[:, kc, :],
                           start=(kc == 0), stop=(kc == KC - 1))
                    qe = evp.tile([P, OWN], f32r, name="qe")
                    nc.scalar.activation(
                        qe[:], ps[:], AF.Identity, bias=bq128[:, m:m + 1]
                    )
                    nc.sync.dma_start(qT_d[m * P:(m + 1) * P, :], qe[:])

            # ---- A-full + B/C: LN1 all tokens -> ln1T; K,V -> kT_d/vT_d ----
            with tc.tile_pool(name="ln1p", bufs=1) as ln1p, \
                 tc.tile_pool(name="worka", bufs=3) as worka, \
                 tc.tile_pool(name="wkvp", bufs=3) as wkvp:
                ln1T = ln1p.tile([P, KC, T], f32r, name="ln1T")
                for i in range(T // P):
                    ln_transpose(x_full, i * P, ln1T, i * P, worka)

                for m in range(KC):   # 8 k row-blocks -> kT_d
                    wkv = wkvp.tile([P, KC * P], f32r, name="wkv")
                    nc.sync.dma_start(wkv[:], wkv_p[m])
                    for nb in range(TB):
                        ps = psum.tile([P, NB], f32, name="ps")
                        for kc in range(KC):
                            mm(ps[:], wkv[:, kc * P:(kc + 1) * P],
                               ln1T[:, kc, nb * NB:(nb + 1) * NB],
                               start=(kc == 0), stop=(kc == KC - 1))
                        ev = evp.tile([P, NB], f32r, name="evr")
                        nc.vector.tensor_scalar_add(
                            ev[:], ps[:], bqkv_t[:, KC + m: KC + m + 1]
                        )
                        nc.sync.dma_start(
                            kT_d[m, :, nb * NB:(nb + 1) * NB], ev[:]
                        )

                # V in [t, d] layout: ln1T slices as lhsT, per-head-contig
                # [t, 16*(64|1)] tiles with ones column -> v65_d. The V GEMM
                # emits the attention lhsT layout for free, so the per-head
                # on-chip re-transposes disappear.
                with tc.tile_pool(name="wvp", bufs=1) as wvp, \
                     tc.tile_pool(name="vstg", bufs=2) as vstg:
                    wv = []
                    for kc in range(KC):
                        t = wvp.tile([P, C], f32r, name=f"wv{kc}")
                        nc.sync.dma_start(t[:], wv_rhs[kc])
                        wv.append(t)
                    for tb in range(T // P):
                        stg = vstg.tile([P, H, D + 1], f32r, name="vstg")
                        for dg in range(2):
                            ps = psum.tile([P, NB], f32, name="ps")
                            for kc in range(KC):
                                mm(ps[:],
                                   ln1T[:, kc, tb * P:(tb + 1) * P],
                                   wv[kc][:, dg * NB:(dg + 1) * NB],
                                   start=(kc == 0), stop=False)
                            mm(ps[:], ones1p[:],
                               bv1[0:1, dg * NB:(dg + 1) * NB],
                               start=False, stop=True)
                            nc.scalar.copy(
                                stg[:, dg * 8:(dg + 1) * 8, 0:D], ps[:]
                            )
                        nc.vector.tensor_copy(
                            stg[:, :, D:D + 1], onesf[:, 0:H]
                        )
                        nc.sync.dma_start(v65_d[tb], stg[:])

            # ---- x1T pool (lives E..H) ----
            x1Tp = ctx2.enter_context(tc.tile_pool(name="x1Tp", bufs=1))
            x1T = [x1Tp.tile([P, OWN], f32, name=f"x1T{j}") for j in range(KC)]

            # ---- E: attention -> aT; F: proj + residual -> x1T ----
            with tc.tile_pool(name="aTp", bufs=1) as aTp:
                aT = [aTp.tile([P, OWN], f32r, name=f"aT{j}")
                      for j in range(KC)]
                with tc.tile_pool(name="maskp", bufs=1) as mpool, \
                     tc.tile_pool(name="kvp", bufs=2) as kvp, \
                     tc.tile_pool(name="v65p", bufs=1) as v65p:
                    v65 = []
                    for tb in range(T // P):
                        t = v65p.tile([P, H, D + 1], f32r, name=f"v65_{tb}")
                        nc.sync.dma_start(t[:], v65_d[tb])
                        v65.append(t)
                    mtiles = {}
                    for i, (qh, sc) in enumerate(MASKED_PAIRS):
                        for sb in range(2):
                            mt = mpool.tile([P, CH], f32r,
                                            name=f"m{qh}_{sc}_{sb}")
                            nc.sync.dma_start(mt[:], mask_in[i, sb])
                            mtiles[(qh, sc, sb)] = mt

                    for h in range(H):
                        qt = kvp.tile([D, OWN], f32r, name="qt", bufs=3)
                        nc.sync.dma_start(qt[:], qT_d[h * D:(h + 1) * D, :])
                        kt = kvp.tile([D, T], f32r, name="kt", bufs=3)
                        nc.sync.dma_start(
                            kt[:], kT_d[h // 2, (h % 2) * D:(h % 2 + 1) * D, :]
                        )
                        for qh in range(2):
                            av = pav.tile([D + 1, CH], f32, name="av")
                            nsc = SRANGE[qh]
                            for sc in range(nsc):
                                masked = (qh, sc) in MASKED_SET
                                ps = psum.tile([P, NB], f32, name="ps")
                                for sb in range(2):
                                    s0 = sc * CH + sb * P
                                    mm(ps[:, sb * CH:(sb + 1) * CH],
                                       kt[:, s0:s0 + P],
                                       qt[:, qh * CH:(qh + 1) * CH],
                                       start=True, stop=not masked)
                                    if masked:
                                        mm(ps[:, sb * CH:(sb + 1) * CH],
                                           id_r[:],
                                           mtiles[(qh, sc, sb)][:],
                                           start=False, stop=True)
                                ex = evp.tile([P, 2 * CH], f32r, name="ex")
                                nc.scalar.activation(ex[:], ps[:], AF.Exp)
                                for sb in range(2):
                                    mm(av[:], v65[sc * 2 + sb][:, h, :],
                                       ex[:, sb * CH:(sb + 1) * CH],
                                       start=(sc == 0 and sb == 0),
                                       stop=(sc == nsc - 1 and sb == 1))
                            rz = stat.tile([1, CH], f32, name="rz")
                            nc.vector.reciprocal(rz[:], av[D:D + 1, :])
                            rzr = stat.tile([1, CH], f32r, name="rzr")
                            nc.scalar.activation(rzr[:], rz[:], AF.Identity)
                            bc = pbc.tile([D, CH], f32, name="bc")
                            mm(bc[:], ones1[:], rzr[:], start=True, stop=True)
                            bcs = evp.tile([D, CH], f32, name="bcs")
                            nc.vector.tensor_copy(bcs[:], bc[:])
                            nc.vector.tensor_mul(
                                aT[h // 2][(h % 2) * D:(h % 2 + 1) * D,
                                           qh * CH:(qh + 1) * CH],
                                av[0:D, :], bcs[:],
                            )

                # ---- F: proj + residual -> x1T ----
                with tc.tile_pool(name="wpp", bufs=3) as wpp:
                    for m in range(KC):
                        wp = wpp.tile([P, KC * P], f32r, name="wp")
                        nc.sync.dma_start(wp[:], wproj_p[m])
                        ps = psum.tile([P, NB], f32, name="ps")
                        for kc in range(KC):
                            mm(ps[:], wp[:, kc * P:(kc + 1) * P], aT[kc][:],
                               start=(kc == 0), stop=(kc == KC - 1))
                        ev = evp.tile([P, NB], f32, name="ev")
                        nc.vector.tensor_scalar_add(
                            ev[:], ps[:], bproj_t[:, m:m + 1]
                        )
                        xo = evp.tile([P, NB], f32, name="xo", tag="ev")
                        nc.sync.dma_start(xo[:], xT_own[m * P:(m + 1) * P, :])
                        nc.vector.tensor_add(x1T[m][:], ev[:], xo[:])

            # ---- F2/G/H: LN2, MLP ----
            with tc.tile_pool(name="hTp", bufs=1) as hTp:
                hT = [hTp.tile([P, OWN], f32r, name=f"hT{j}")
                      for j in range(FB)]
                with tc.tile_pool(name="ln2p", bufs=1) as ln2p, \
                     tc.tile_pool(name="workf", bufs=3) as workf, \
                     tc.tile_pool(name="w1p", bufs=3) as w1p:
                    ln2T = ln2p.tile([P, KC, OWN], f32r, name="ln2T")
                    for i in range(OWN // P):
                        x1 = workf.tile([P, C], f32, name="xt")
                        for g in range(KC // 4):
                            ps = pst.tile([P, 4, P], f32, name="tps")
                            for jj in range(4):
                                j = 4 * g + jj
                                nc.tensor.transpose(
                                    ps[:, jj, :],
                                    x1T[j][:, i * P:(i + 1) * P],
                                    identity[:],
                                )
                            nc.scalar.copy(
                                x1[:, 4 * g * P:4 * (g + 1) * P], ps[:]
                            )
                        z = workf.tile([P, C], f32, name="z")
                        layernorm_tile(x1, z, workf)
                        transpose_to(ln2T, z, i * P)

                    for m in range(FB):
                        w1 = w1p.tile([P, KC * P], f32r, name="w1")
                        nc.sync.dma_start(w1[:], wl1_p[m])
                        ps = psum.tile([P, NB], f32, name="ps")
                        for kc in range(KC):
                            mm(ps[:], w1[:, kc * P:(kc + 1) * P],
                               ln2T[:, kc, :],
                               start=(kc == 0), stop=(kc == KC - 1))
                        nc.scalar.activation(hT[m][:], ps[:], AF.Relu,
                                             bias=bl1_t[:, m:m + 1])

                with tc.tile_pool(name="w3p", bufs=2) as w3p:
                    for m in range(KC):
                        w3 = w3p.tile([P, FB * P], f32r, name="w3")
                        nc.sync.dma_start(w3[:], wl3_p[m])
                        ps = psum.tile([P, NB], f32, name="ps")
                        for fc in range(FB):
                            mm(ps[:], w3[:, fc * P:(fc + 1) * P], hT[fc][:],
                               start=(fc == 0), stop=(fc == FB - 1))
                        ev = evp.tile([P, NB], f32, name="ev")
                        nc.vector.tensor_scalar_add(
                            ev[:], ps[:], bl3_t[:, m:m + 1]
                        )
                        o = evp.tile([P, NB], f32, name="o", tag="ev")
                        nc.vector.tensor_add(o[:], ev[:], x1T[m][:])
                        nc.sync.dma_start(outT[m * P:(m + 1) * P, :], o[:])

        if n_iters == 1:
            with ExitStack() as ctx2:
                body(ctx2)
        else:
            with tc.For_i(0, n_iters, 1):
                with ExitStack() as ctx2:
                    body(ctx2)

    return nc


def _pack(wT, nblk, blk, kblk=P):
    """Pack lhsT source wT [K_total, M_total] into [nblk, P, (K/P)*blk]
    tiles: packed[m, p, kc*blk + j] = wT[kc*P + p, m*blk + j]."""
    K_total, M_total = wT.shape
    kc = K_total // P
    assert M_total == nblk * blk
    return np.ascontiguousarray(
        wT.reshape(kc, P, nblk, blk).transpose(2, 1, 0, 3)
        .reshape(nblk, P, kc * blk)
    )


def _host_prep(x, qkv_w, proj_w, proj_b, l1_w, l1_b, l3_w, l3_b,
               ln1_g, ln1_b, ln2_g, ln2_b):
    f = np.float32
    x = np.asarray(x, f)
    qkv_w = np.asarray(qkv_w, f)
    scale = np.float32(D ** -0.5)
    w_eff = qkv_w * np.asarray(ln1_g, f)[None, :]
    b_eff = (qkv_w @ np.asarray(ln1_b, f)).astype(f)
    w_eff[:C] *= scale
    b_eff[:C] *= scale
    l1_eff = np.asarray(l1_w, f) * np.asarray(ln2_g, f)[None, :]
    bl1_eff = (np.asarray(l1_b, f)
               + np.asarray(l1_w, f) @ np.asarray(ln2_b, f)).astype(f)
    wT = np.ascontiguousarray(w_eff.T)       # [cin, 3C]
    shared = {
        "wq_p": _pack(wT[:, 0:C], H // 2, P),
        "wkv_p": _pack(wT[:, C:2 * C], KC, P),
        "wv_rhs": np.ascontiguousarray(wT[:, 2 * C:3 * C].reshape(KC, P, C)),
        "wproj_p": _pack(np.ascontiguousarray(np.asarray(proj_w, f).T),
                         KC, P),
        "wl1_p": _pack(np.ascontiguousarray(l1_eff.T), FB, P),
        "wl3_p": _pack(np.ascontiguousarray(np.asarray(l3_w, f).T), KC, P),
        "bqkv": b_eff,
        "bproj": np.asarray(proj_b, f),
        "bl1": bl1_eff,
        "bl3": np.asarray(l3_b, f),
    }

    in_maps = []
    for cid in range(N_CORES):
        b, r = divmod(cid, RANKS)
        lo, hi = r, NCHUNK - 1 - r
        own_idx = np.r_[lo * CH:(lo + 1) * CH, hi * CH:(hi + 1) * CH]
        xb = x[b]
        x_own = np.ascontiguousarray(xb[own_idx])
        # additive mask: 0 where the (s, q) pair is live, -30000 where not
        m = np.zeros((len(MASKED_PAIRS), 2, P, CH), f)
        tri0 = (np.arange(P)[:, None] <= np.arange(CH)[None, :]).astype(f)
        tri1 = (np.arange(P)[:, None] + P <= np.arange(CH)[None, :]).astype(f)
        for i, (qh, sc) in enumerate(MASKED_PAIRS):
            qc = lo if qh == 0 else hi
            if sc < qc:
                m[i] = 1.0
            elif sc == qc:
                m[i, 0] = tri0
                m[i, 1] = tri1
        m = (m - 1.0) * 30000.0
        in_maps.append({
            "x_full": np.ascontiguousarray(xb),
            "x_own": x_own,
            "xT_own": np.ascontiguousarray(x_own.T),
            "mask": m,
            **shared,
        })
    return in_maps


def _assemble(results):
    out = np.empty((B, T, C), np.float32)
    for cid in range(N_CORES):
        b, r = divmod(cid, RANKS)
        lo, hi = r, NCHUNK - 1 - r
        oT = results[cid]["outT"]
        out[b, lo * CH:(lo + 1) * CH] = oT[:, 0:CH].T
        out[b, hi * CH:(hi + 1) * CH] = oT[:, CH:2 * CH].T
    return out


_CACHE = {}


def get_nc(n_iters=1):
    if n_iters not in _CACHE:
        import concourse.bacc as bacc
        import concourse.tile as tile
        from concourse import mybir
        nc = bacc.Bacc("TRN2", target_bir_lowering=False, debug=False,
                       num_devices=N_CORES)
        build_core_program(nc, tile, mybir, n_iters=n_iters)
        nc.compile()
        _CACHE[n_iters] = nc
    return _CACHE[n_iters]


def run(inputs, n_iters=1):
    from concourse.bass_utils import run_bass_kernel_spmd
    in_maps = _host_prep(**inputs)
    nc = get_nc(n_iters)
    res = run_bass_kernel_spmd(nc, in_maps, list(range(N_CORES)))
    return _assemble(res.results)


def kernel(**inputs):
    return run(inputs, n_iters=1)

